# revision 32
# baseline (speedup 1.0000x reference)
"""Multi-head attention (ReLU-gated projections) on 8 Trainium2 NeuronCores.

Problem (hardcoded): B=4, S=1024, H=1024, NH=16, DH=64.
  qp = relu(q @ Wq.T + bq); kp, vp likewise
  alpha = softmax(qh @ kh.T / sqrt(DH)) * mask[q]
  out = (alpha @ vh).reshape(B,S,H) + query

Sharding: 8 cores = 4 batches x 2 head-groups (8 heads / 512 hidden cols each).

fp8 design (per core):
  - inputs x/W quantized host-side to fp8e4m3 (TRN float8e4, max 240).
  - projections as fp8 DoubleRow matmuls (2x contraction per cycle):
    qp/kp evac'd with fused bias+relu to bf16, vp to fp8 (with a ones column
    per head so AV accumulates sumexp for free, plus one pad column so the
    DoubleRow pair stride is 16B-aligned: 66 cols/head).
  - alpha: bf16 K=64 matmuls, two heads concurrently on disjoint 64-row
    PE row-groups (2x row tiling; tile_position auto-derived from
    base_partition of the kp/qp slices).
  - softmax exp with a global shift: pt = exp(alpha*SCALE - SHIFT) so pt fits
    fp8e4m3 range; numerator and denominator scale together so the softmax is
    exact.  exp split across two engines: ACT (spline exp -> fp8 out) and DVE
    (Schraudolph: k8 = (alpha*A + B) as int8, bitcast to fp8e4m3).
  - AV: fp8 DoubleRow (vp pairs of k-chunks stationary, pt pairs moving),
    av PSUM [66,512] DMA'd directly to DRAM (rows 0-64; row 64 = sumexp).
  - host divides by sumexp, applies mask, adds residual.
"""
import sys

sys.path.insert(0, "/opt/trn_rl_repo")

import math
import os
import numpy as np
import ml_dtypes

import concourse.bass as bass
import concourse.tile as tile
from concourse import bacc, mybir
from concourse import bass_utils

if os.environ.get("BASS_LDW_OPT", "0") == "1":
    _orig_run_command = bass_utils.run_command

    def _patched_run_command(cmd, **kw):
        cmd = ["--enable-ldw-opt=true" if c == "--enable-ldw-opt=false" else c
               for c in cmd]
        return _orig_run_command(cmd, **kw)

    bass_utils.run_command = _patched_run_command

B, S, H = 4, 1024, 1024
NH, DH = 16, 64
NCORES = 8
GROUPS = 2          # head-groups (tensor-parallel dim)
HL = NH // GROUPS   # heads per core = 8
GH = H // GROUPS    # hidden cols per core = 512
KT = H // 128       # contraction k-tiles = 8
OT = GH // 128      # output o-tiles per core = 4
SCALE = 1.0 / float(np.sqrt(DH))
SHIFT = 4.0         # global exp shift: pt = exp(alpha*SCALE - SHIFT)
PVW = DH + 2        # padded per-head v width (64 v + 1 ones + 1 pad) = 66
VW = HL * PVW       # v cols per k-chunk = 528 (16B aligned)

# Schraudolph constants for fp8e4m3 (bias 7, 3 mantissa bits):
#   k8 = (alpha * SCALE - SHIFT) * 8/ln2 + 56 - c ; bitcast int8 -> fp8
_LN2 = math.log(2.0)
SCHR_C = float(os.environ.get("BASS_SCHR_C", "0.45"))
SCHR_A = SCALE * 8.0 / _LN2
# +0.5: DVE f32->int8 convert truncates (matches CoreSim); makes it rounding.
SCHR_B = 56.0 - SCHR_C - SHIFT * 8.0 / _LN2 \
    + float(os.environ.get("BASS_SCHR_HALF", "0.5"))

MODE = os.environ.get("BASS_MM_DT", "fp8")
WARM = int(os.environ.get("BASS_WARM", "6"))
ACT_EXPS = int(os.environ.get("BASS_ACT_EXPS", "32"))  # of 64 exp tiles

F32 = mybir.dt.float32
BF16 = mybir.dt.bfloat16
FP8 = mybir.dt.float8e4
I8 = mybir.dt.int8
DR = mybir.MatmulPerfMode.DoubleRow
E4NP = ml_dtypes.float8_e4m3


def build(mode, bias_v=False):
    assert mode == "fp8"
    nc = bacc.Bacc("TRN2", target_bir_lowering=False, debug=False,
                   num_devices=NCORES)

    # x: [p, sc(2), k(8), s'(512)] -> [128, 8192]; h = k*128+p, s = sc*512+s'
    xq_d = nc.dram_tensor("xq", [128, 2 * KT * 512], FP8,
                          kind="ExternalInput").ap()
    xk_d = nc.dram_tensor("xk", [128, 2 * KT * 512], FP8,
                          kind="ExternalInput").ap()
    xv_d = nc.dram_tensor("xv", [128, 2 * KT * 512], FP8,
                          kind="ExternalInput").ap()
    # wq/wk: [p, ot(4), k(8), o'(128)] -> [128, 4096]
    wq_d = nc.dram_tensor("wq", [128, OT * KT * 128], FP8,
                          kind="ExternalInput").ap()
    wk_d = nc.dram_tensor("wk", [128, OT * KT * 128], FP8,
                          kind="ExternalInput").ap()
    # wv: [p, k(8), o(512)] -> [128, 4096]
    wv_d = nc.dram_tensor("wv", [128, KT * GH], FP8, kind="ExternalInput").ap()
    bqk_d = nc.dram_tensor("bqk", [128, 2 * OT], F32, kind="ExternalInput").ap()
    bv_d = nc.dram_tensor("bv", [1, GH], FP8, kind="ExternalInput").ap()
    hid_d = nc.dram_tensor("hid", [HL * (DH + 1), S], BF16,
                           kind="ExternalOutput").ap()

    with tile.TileContext(nc) as tc:
        with tc.tile_pool(name="sb", bufs=1) as sb, \
             tc.tile_pool(name="ps", bufs=1, space="PSUM") as ps:

            # ---- persistent SBUF tiles (one per DMA for fine-grain deps) ----
            HK = KT // 2 * 512  # 2048 cols per (sc, k-half)
            x_t = {}
            for which in ("q", "k", "v"):
                for sc in (0, 1):
                    for kh in (0, 1):
                        nm = f"x{which}{sc}{kh}"
                        x_t[(which, sc, kh)] = sb.tile(
                            [128, HK], FP8, tag=nm, name=nm)
            wq_t = [sb.tile([128, 2 * KT * 128], FP8, tag=f"wq{i}",
                            name=f"wq{i}") for i in range(2)]
            wk_t = [sb.tile([128, 2 * KT * 128], FP8, tag=f"wk{i}",
                            name=f"wk{i}") for i in range(2)]
            wv_t = [sb.tile([128, KT // 2 * GH], FP8, tag=f"wv{i}",
                            name=f"wv{i}") for i in range(2)]
            qp_t = [sb.tile([128, S], BF16, tag=f"qp{t}", name=f"qp{t}")
                    for t in range(OT)]
            kp_t = [sb.tile([128, S], BF16, tag=f"kp{t}", name=f"kp{t}")
                    for t in range(OT)]
            vp_t = sb.tile([128, KT * VW], FP8, tag="vp", name="vp")
            bqk_t = sb.tile([128, 2 * OT], F32, tag="bqk", name="bqk")
            bv_t = sb.tile([1, GH], FP8, tag="bv", name="bv")
            wa_t = sb.tile([128, 128], FP8, tag="wa", name="wa")
            wb_t = sb.tile([128, 512], FP8, tag="wb", name="wb")
            dummy_t = sb.tile([1, 8], F32, tag="dummy", name="dummy")
            nshift_t = sb.tile([128, 1], F32, tag="nshift", name="nshift")

            # ---- t=0: DMA-free warmup (memset inputs) + const setup ----
            nc.vector.memset(wa_t[:], 1.0)
            nc.vector.memset(wb_t[:], 0.0)
            nc.gpsimd.memset(nshift_t[:], -SHIFT)
            # vp ones + pad columns (per head, per k-chunk)
            vp4 = vp_t[:].rearrange("p (k n c) -> p k n c", n=HL, c=PVW)
            nc.vector.memset(vp4[:, :, :, DH:DH + 1], 1.0)
            nc.vector.memset(vp4[:, :, :, DH + 1:DH + 2], 0.0)
            for i in range(WARM):
                warm = ps.tile([128, 512], F32, tag="small", bufs=2,
                               name=f"warm{i}")
                nc.tensor.matmul(warm[:], wa_t[:], wb_t[:],
                                 start=True, stop=True)

            # ---- input DMAs: k-half quarters, priority q > k > v ----
            def xq4(which, sc, kh):
                xd = {"q": xq_d, "k": xk_d, "v": xv_d}[which]
                dst = x_t[(which, sc, kh)][:]
                src = xd[:, sc * KT * 512 + kh * HK:
                         sc * KT * 512 + (kh + 1) * HK]
                return dst, src

            # Pin each input DMA's *scheduling-pass* timestamp to its
            # realistic completion time (the scheduler's DMA model is
            # optimistic, which otherwise orders consumers of late DMAs
            # ahead of ready work in the engine FIFOs).
            def pdma(eng, dst, src, ms):
                with tc.tile_wait_until(ms):
                    eng.dma_start(dst, src)

            pdma(nc.gpsimd, wq_t[0][:], wq_d[:, 0:2 * KT * 128], 0.0025)
            pdma(nc.gpsimd, wk_t[0][:], wk_d[:, 0:2 * KT * 128], 0.004)
            pdma(nc.gpsimd, bqk_t[:], bqk_d, 0.0045)
            pdma(nc.gpsimd, bv_t[:], bv_d, 0.0046)
            pdma(nc.gpsimd, wq_t[1][:], wq_d[:, 2 * KT * 128:], 0.0065)
            pdma(nc.gpsimd, wk_t[1][:], wk_d[:, 2 * KT * 128:], 0.008)
            xq_ms = {("q", 0): 0.004, ("q", 1): 0.005,
                     ("k", 0): 0.0065, ("k", 1): 0.008}
            for which in ("q", "k"):
                for sc in (0, 1):
                    ms = xq_ms[(which, sc)]
                    pdma(nc.sync, *xq4(which, sc, 0), ms)
                    pdma(nc.scalar, *xq4(which, sc, 1), ms)
            pdma(nc.sync, *xq4("v", 0, 0), 0.012)
            pdma(nc.scalar, *xq4("v", 1, 0), 0.012)
            pdma(nc.gpsimd, *xq4("v", 0, 1), 0.0125)
            pdma(nc.gpsimd, *xq4("v", 1, 1), 0.013)
            pdma(nc.sync, wv_t[0][:], wv_d[:, 0:KT // 2 * GH], 0.0135)
            pdma(nc.scalar, wv_t[1][:], wv_d[:, KT // 2 * GH:], 0.0135)
            # preload ACT exp table (after the scalar-ring DMA kicks)
            nc.scalar.activation(dummy_t[:], wb_t[0:1, 0:8],
                                 mybir.ActivationFunctionType.Exp, scale=1.0)

            # rearranged views: (which, sc, c2) -> moving/stationary k-pair
            def wqk_pair(w_t, ot, c2):
                half = w_t[ot // 2]
                base = (ot % 2) * KT * 128
                return half[:, base:base + KT * 128].rearrange(
                    "p (k o) -> p k o", o=128)[:, 2 * c2:2 * c2 + 2, :]

            def x_pair(which, sc, c2):
                kh, c = c2 // 2, c2 % 2
                return x_t[(which, sc, kh)][:].rearrange(
                    "p (k s) -> p k s", s=512)[:, 2 * c:2 * c + 2, :]

            def wv_pair(c2):
                kh, c = c2 // 2, c2 % 2
                return wv_t[kh][:].rearrange(
                    "p (k o) -> p k o", o=GH)[:, 2 * c:2 * c + 2, :]

            vp3 = vp_t[:].rearrange("p (k m) -> p k m", m=VW)

            # ---- engine balancing for exp tiles ----
            exp_state = {"acc": 0}

            def exp_engine():
                exp_state["acc"] += ACT_EXPS
                if exp_state["acc"] >= 64:
                    exp_state["acc"] -= 64
                    return "act"
                return "dve"

            pt_tiles = {}

            def pt_tile(n, c):
                if (n, c) not in pt_tiles:
                    pt_tiles[(n, c)] = sb.tile([128, 2048], FP8, tag="pt",
                                               bufs=32, name=f"pt_{n}_{c}")
                return pt_tiles[(n, c)]

            # ---- stage helpers ----
            def proj_qk(which, ot):
                w_t = wq_t if which == "q" else wk_t
                dst = qp_t[ot] if which == "q" else kp_t[ot]
                wi = 0 if which == "q" else 1
                bias = bqk_t[:, wi * OT + ot:wi * OT + ot + 1]
                pp = ps.tile([128, 1024], F32, tag="apt", bufs=3,
                             name=f"pp{which}{ot}")
                for sc in range(2):
                    for c2 in range(KT // 2):
                        nc.tensor.matmul(
                            pp[:, sc * 512:(sc + 1) * 512],
                            wqk_pair(w_t, ot, c2),
                            x_pair(which, sc, c2),
                            start=(c2 == 0), stop=(c2 == KT // 2 - 1),
                            perf_mode=DR)
                nc.scalar.activation(
                    dst[:], pp[:],
                    mybir.ActivationFunctionType.Relu,
                    bias=bias, scale=1.0)

            def proj_v(st):
                sc, j = st // 4, st % 4
                pp = ps.tile([128, 512], F32, tag="small", bufs=2,
                             name=f"ppv{st}")
                if bias_v:
                    nc.tensor.matmul(pp[:], wa_t[0:1, :], bv_t[:],
                                     start=True, stop=False)
                for c2 in range(KT // 2):
                    nc.tensor.matmul(
                        pp[:],
                        x_pair("v", sc, c2)[:, :, j * 128:(j + 1) * 128],
                        wv_pair(c2),
                        start=(c2 == 0 and not bias_v),
                        stop=(c2 == KT // 2 - 1),
                        perf_mode=DR)
                # evac with relu into the strided fp8 v layout (cols 0..63)
                vdst = vp4[:, st, :, 0:DH]
                psrc = pp[:].rearrange("p (n c) -> p n c", c=DH)
                nc.vector.tensor_scalar(
                    vdst, psrc, 0.0, None, mybir.AluOpType.max)

            def alpha_pair(t, k):
                """alpha + exp for heads (2t, 2t+1), sk-tile k: two K=64
                matmuls on disjoint PE row-groups run concurrently."""
                apts = []
                for h in range(2):
                    apt = ps.tile([128, 1024], F32, tag="apt", bufs=3,
                                  name=f"alp_{2 * t + h}_{k}")
                    apts.append(apt)
                for h in range(2):
                    for qc in range(2):
                        pr = slice(h * 64, h * 64 + 64)
                        nc.tensor.matmul(
                            apts[h][:, qc * 512:(qc + 1) * 512],
                            kp_t[t][pr, k * 128:(k + 1) * 128],
                            qp_t[t][pr, qc * 512:(qc + 1) * 512],
                            start=True, stop=True)
                for h in range(2):
                    n = 2 * t + h
                    pt = pt_tile(n, k // 2)
                    half = pt[:, (k % 2) * 1024:(k % 2) * 1024 + 1024]
                    if exp_engine() == "act":
                        nc.scalar.activation(
                            half, apts[h][:],
                            mybir.ActivationFunctionType.Exp,
                            bias=nshift_t[:], scale=SCALE)
                    else:
                        nc.vector.tensor_scalar(
                            half.bitcast(I8), apts[h][:],
                            SCHR_A, SCHR_B,
                            mybir.AluOpType.mult, mybir.AluOpType.add)

            av_state = {"i": 0}
            hs_tiles = {}

            def av_qc(n, qc):
                if n not in hs_tiles:
                    hs_tiles[n] = sb.tile([DH + 1, S], BF16, tag="hid",
                                          bufs=3, name=f"hid_{n}")
                hs = hs_tiles[n]
                av = ps.tile([128, 512], F32, tag="small", bufs=2,
                             name=f"av_{n}_{qc}")
                for c2 in range(KT // 2):
                    pt = pt_tile(n, c2)
                    ptm = pt[:].rearrange("p (two q) -> p two q", two=2)
                    nc.tensor.matmul(
                        av[0:PVW, :],
                        vp3[:, 2 * c2:2 * c2 + 2,
                            n * PVW:(n + 1) * PVW],
                        ptm[:, :, qc * 512:(qc + 1) * 512],
                        start=(c2 == 0), stop=(c2 == KT // 2 - 1),
                        perf_mode=DR)
                i = av_state["i"]
                av_state["i"] += 1
                dst = hs[:, qc * 512:(qc + 1) * 512]
                if i % 2 == 0:
                    nc.scalar.copy(dst, av[0:DH + 1, :])
                else:
                    nc.vector.tensor_copy(dst, av[0:DH + 1, :])
                eng = nc.sync if i % 2 == 0 else nc.gpsimd
                eng.dma_start(
                    hid_d[n * (DH + 1):(n + 1) * (DH + 1),
                          qc * 512:(qc + 1) * 512],
                    dst)

            # ---- emission schedule ----
            # Phase A: all q/k projections (one contiguous DR stream,
            #          DMA-paced).  Phase B: one long exp-paced alpha
            #          stream with a few large DR filler blocks (proj_v
            #          pairs, early av groups) — few tiling-mode switches.
            # Phase C: remaining AV + output.
            def at(ms, fn, *args):
                with tc.tile_wait_until(ms):
                    fn(*args)

            proj_qk("q", 0)
            proj_qk("k", 0)
            proj_qk("q", 1)
            proj_qk("k", 1)
            at(0.0090, proj_qk, "q", 2)
            at(0.0095, proj_qk, "k", 2)
            at(0.0100, proj_qk, "q", 3)
            at(0.0105, proj_qk, "k", 3)

            clk = [0.0125]
            DS = 0.00118          # one alpha k-step (2 apts at exp pace)

            def alpha_step(t, k):
                at(clk[0], alpha_pair, t, k)
                clk[0] += DS

            def fill_pv(st0):
                at(clk[0], proj_v, st0)
                at(clk[0] + 0.0004, proj_v, st0 + 1)
                clk[0] += 0.0018

            def fill_av(n):
                at(clk[0], av_qc, n, 0)
                at(clk[0] + 0.0004, av_qc, n, 1)
                clk[0] += 0.0018

            for k in range(8):
                alpha_step(0, k)
            fill_pv(0)
            for k in range(4):
                alpha_step(1, k)
            fill_pv(2)
            for k in range(4, 8):
                alpha_step(1, k)
            fill_pv(4)
            for k in range(4):
                alpha_step(2, k)
            fill_pv(6)
            for k in range(4, 8):
                alpha_step(2, k)
            for k in range(4):
                alpha_step(3, k)
            fill_av(0)
            for k in range(4, 8):
                alpha_step(3, k)
            fill_av(1)
            for n in range(2, 8):
                fill_av(n)

    nc.compile()
    return nc


_NC_CACHE = {}


def _get_nc(mode, bias_v=False):
    key = (mode, bias_v)
    if key not in _NC_CACHE:
        _NC_CACHE[key] = build(mode, bias_v)
    return _NC_CACHE[key]


def _prep_inputs(inputs, mode):
    q = np.asarray(inputs["query"], np.float32)
    k = np.asarray(inputs["key"], np.float32)
    v = np.asarray(inputs["value"], np.float32)
    Wq = np.asarray(inputs["Wq"], np.float32)
    Wk = np.asarray(inputs["Wk"], np.float32)
    Wv = np.asarray(inputs["Wv"], np.float32)
    bq = np.asarray(inputs["bq"], np.float32)
    bk = np.asarray(inputs["bk"], np.float32)
    bv = np.asarray(inputs["bv"], np.float32)

    def xprep(x, b):
        # [H, S] -> [p, sc, k, s'] -> [128, 8192] fp8
        xt = np.ascontiguousarray(x[b].T).astype(E4NP)
        return np.ascontiguousarray(
            xt.reshape(KT, 128, 2, 512).transpose(1, 2, 0, 3)
        ).reshape(128, 2 * KT * 512)

    def wqk_prep(W, sl):
        # W[sl].T: [H, GH] -> [p, ot, k, o'] -> [128, 4096] fp8
        wt = np.ascontiguousarray(W[sl, :].T).astype(E4NP)
        return np.ascontiguousarray(
            wt.reshape(KT, 128, OT, 128).transpose(1, 2, 0, 3)
        ).reshape(128, OT * KT * 128)

    def wv_prep(W, sl):
        # W[sl].T: [H, GH] -> [p, k, o] -> [128, 4096] fp8
        wt = np.ascontiguousarray(W[sl, :].T).astype(E4NP)
        return np.ascontiguousarray(
            wt.reshape(KT, 128, GH).transpose(1, 0, 2)
        ).reshape(128, KT * GH)

    xq = [xprep(q, b) for b in range(B)]
    xk = [xprep(k, b) for b in range(B)]
    xv = [xprep(v, b) for b in range(B)]
    in_maps = []
    for c in range(NCORES):
        b, g = c // GROUPS, c % GROUPS
        sl = slice(g * GH, (g + 1) * GH)
        bqk = np.stack([bq[sl].reshape(OT, 128).T,
                        bk[sl].reshape(OT, 128).T], 1).reshape(128, 2 * OT)
        in_maps.append({
            "xq": xq[b], "xk": xk[b], "xv": xv[b],
            "wq": wqk_prep(Wq, sl),
            "wk": wqk_prep(Wk, sl),
            "wv": wv_prep(Wv, sl),
            "bqk": np.ascontiguousarray(bqk, dtype=np.float32),
            "bv": np.ascontiguousarray(bv[None, sl]).astype(E4NP),
        })
    return in_maps


def run(inputs, mode=MODE, trace=False):
    bias_v = bool(np.any(np.asarray(inputs["bv"], np.float32)))
    nc = _get_nc(mode, bias_v)
    in_maps = _prep_inputs(inputs, mode)
    res = bass_utils.run_bass_kernel_spmd(
        nc, in_maps, core_ids=list(range(NCORES)), trace=trace)

    masks = np.asarray(inputs["masks"], np.float32)
    query = np.asarray(inputs["query"], np.float32)
    out = np.empty((B, S, H), np.float32)
    for c in range(NCORES):
        b, g = c // GROUPS, c % GROUPS
        hid = np.asarray(res.results[c]["hid"],
                         dtype=np.float32).reshape(HL, DH + 1, S)
        hT = hid[:, :DH, :]                      # (HL, DH, S)
        se = hid[:, DH, :]                       # (HL, S)
        blk = (hT / se[:, None, :]).transpose(2, 0, 1).reshape(S, GH)
        out[b, :, g * GH:(g + 1) * GH] = blk
    out = out * masks[:, :, None] + query
    return out, res


def kernel(**inputs) -> np.ndarray:
    out, _ = run(inputs)
    return out


# revision 34
# speedup vs baseline: 1.1228x; 1.1228x over previous
"""Multi-head attention (ReLU-gated projections) on 8 Trainium2 NeuronCores.

Problem (hardcoded): B=4, S=1024, H=1024, NH=16, DH=64.
  qp = relu(q @ Wq.T + bq); kp, vp likewise
  alpha = softmax(qh @ kh.T / sqrt(DH)) * mask[q]
  out = (alpha @ vh).reshape(B,S,H) + query

Sharding: 8 cores = 4 batches x 2 head-groups (8 heads / 512 hidden cols each).

fp8 design (per core):
  - inputs x/W quantized host-side to fp8e4m3 (TRN float8e4, max 240).
  - projections as fp8 DoubleRow matmuls (2x contraction per cycle):
    qp/kp evac'd with fused bias+relu to bf16, vp to fp8 (with a ones column
    per head so AV accumulates sumexp for free, plus one pad column so the
    DoubleRow pair stride is 16B-aligned: 66 cols/head).
  - alpha: bf16 K=64 matmuls, two heads concurrently on disjoint 64-row
    PE row-groups (2x row tiling; tile_position auto-derived from
    base_partition of the kp/qp slices).
  - softmax exp with a global shift: pt = exp(alpha*SCALE - SHIFT) so pt fits
    fp8e4m3 range; numerator and denominator scale together so the softmax is
    exact.  exp split across two engines: ACT (spline exp -> fp8 out) and DVE
    (Schraudolph: k8 = (alpha*A + B) as int8, bitcast to fp8e4m3).
  - AV: fp8 DoubleRow (vp pairs of k-chunks stationary, pt pairs moving),
    av PSUM [66,512] DMA'd directly to DRAM (rows 0-64; row 64 = sumexp).
  - host divides by sumexp, applies mask, adds residual.
"""
import sys

sys.path.insert(0, "/opt/trn_rl_repo")

import math
import os
import numpy as np
import ml_dtypes

import concourse.bass as bass
import concourse.tile as tile
from concourse import bacc, mybir
from concourse import bass_utils

if os.environ.get("BASS_LDW_OPT", "0") == "1":
    _orig_run_command = bass_utils.run_command

    def _patched_run_command(cmd, **kw):
        cmd = ["--enable-ldw-opt=true" if c == "--enable-ldw-opt=false" else c
               for c in cmd]
        return _orig_run_command(cmd, **kw)

    bass_utils.run_command = _patched_run_command

B, S, H = 4, 1024, 1024
NH, DH = 16, 64
NCORES = 8
GROUPS = 2          # head-groups (tensor-parallel dim)
HL = NH // GROUPS   # heads per core = 8
GH = H // GROUPS    # hidden cols per core = 512
KT = H // 128       # contraction k-tiles = 8
OT = GH // 128      # output o-tiles per core = 4
SCALE = 1.0 / float(np.sqrt(DH))
SHIFT = 4.0         # global exp shift: pt = exp(alpha*SCALE - SHIFT)
PVW = DH + 2        # padded per-head v width (64 v + 1 ones + 1 pad) = 66
VW = HL * PVW       # v cols per k-chunk = 528 (16B aligned)

# Schraudolph constants for fp8e4m3 (bias 7, 3 mantissa bits):
#   k8 = (alpha * SCALE - SHIFT) * 8/ln2 + 56 - c ; bitcast int8 -> fp8
_LN2 = math.log(2.0)
SCHR_C = float(os.environ.get("BASS_SCHR_C", "0.45"))
SCHR_A = SCALE * 8.0 / _LN2
# +0.5: DVE f32->int8 convert truncates (matches CoreSim); makes it rounding.
SCHR_B = 56.0 - SCHR_C - SHIFT * 8.0 / _LN2 \
    + float(os.environ.get("BASS_SCHR_HALF", "0.5"))

MODE = os.environ.get("BASS_MM_DT", "fp8")
WARM = int(os.environ.get("BASS_WARM", "12"))
ACT_EXPS = int(os.environ.get("BASS_ACT_EXPS", "32"))  # of 64 exp tiles

F32 = mybir.dt.float32
BF16 = mybir.dt.bfloat16
FP8 = mybir.dt.float8e4
I8 = mybir.dt.int8
DR = mybir.MatmulPerfMode.DoubleRow
E4NP = ml_dtypes.float8_e4m3


def build(mode, bias_v=False):
    assert mode == "fp8"
    nc = bacc.Bacc("TRN2", target_bir_lowering=False, debug=False,
                   num_devices=NCORES)

    # x: [p, sc(2), k(8), s'(512)] -> [128, 8192]; h = k*128+p, s = sc*512+s'
    xq_d = nc.dram_tensor("xq", [128, 2 * KT * 512], FP8,
                          kind="ExternalInput").ap()
    xk_d = nc.dram_tensor("xk", [128, 2 * KT * 512], FP8,
                          kind="ExternalInput").ap()
    xv_d = nc.dram_tensor("xv", [128, 2 * KT * 512], FP8,
                          kind="ExternalInput").ap()
    # wq/wk: [p, ot(4), k(8), o'(128)] -> [128, 4096]
    wq_d = nc.dram_tensor("wq", [128, OT * KT * 128], FP8,
                          kind="ExternalInput").ap()
    wk_d = nc.dram_tensor("wk", [128, OT * KT * 128], FP8,
                          kind="ExternalInput").ap()
    # wv: [p, k(8), o(512)] -> [128, 4096]
    wv_d = nc.dram_tensor("wv", [128, KT * GH], FP8, kind="ExternalInput").ap()
    bqk_d = nc.dram_tensor("bqk", [128, 2 * OT], F32, kind="ExternalInput").ap()
    bv_d = nc.dram_tensor("bv", [1, GH], FP8, kind="ExternalInput").ap()
    hid_d = nc.dram_tensor("hid", [HL * (DH + 1), S], BF16,
                           kind="ExternalOutput").ap()

    with tile.TileContext(nc) as tc:
        with tc.tile_pool(name="sb", bufs=1) as sb, \
             tc.tile_pool(name="ps", bufs=1, space="PSUM") as ps:

            # ---- persistent SBUF tiles (one per DMA for fine-grain deps) ----
            HK = KT // 2 * 512  # 2048 cols per (sc, k-half)
            x_t = {}
            for which in ("q", "k", "v"):
                for sc in (0, 1):
                    for kh in (0, 1):
                        nm = f"x{which}{sc}{kh}"
                        x_t[(which, sc, kh)] = sb.tile(
                            [128, HK], FP8, tag=nm, name=nm)
            wq_t = [sb.tile([128, 2 * KT * 128], FP8, tag=f"wq{i}",
                            name=f"wq{i}") for i in range(2)]
            wk_t = [sb.tile([128, 2 * KT * 128], FP8, tag=f"wk{i}",
                            name=f"wk{i}") for i in range(2)]
            wv_t = [sb.tile([128, KT // 2 * GH], FP8, tag=f"wv{i}",
                            name=f"wv{i}") for i in range(2)]
            qp_t = [sb.tile([128, S], BF16, tag=f"qp{t}", name=f"qp{t}")
                    for t in range(OT)]
            kp_t = [sb.tile([128, S], BF16, tag=f"kp{t}", name=f"kp{t}")
                    for t in range(OT)]
            vp_t = sb.tile([128, KT * VW], FP8, tag="vp", name="vp")
            bqk_t = sb.tile([128, 2 * OT], F32, tag="bqk", name="bqk")
            bv_t = sb.tile([1, GH], FP8, tag="bv", name="bv")
            wa_t = sb.tile([128, 128], FP8, tag="wa", name="wa")
            wb_t = sb.tile([128, 512], FP8, tag="wb", name="wb")
            dummy_t = sb.tile([1, 8], F32, tag="dummy", name="dummy")
            nshift_t = sb.tile([128, 1], F32, tag="nshift", name="nshift")

            # ---- t=0: DMA-free warmup (memset inputs) + const setup ----
            nc.vector.memset(wa_t[:], 1.0)
            nc.vector.memset(wb_t[:], 0.0)
            nc.gpsimd.memset(nshift_t[:], -SHIFT)
            # vp ones + pad columns (per head, per k-chunk)
            vp4 = vp_t[:].rearrange("p (k n c) -> p k n c", n=HL, c=PVW)
            nc.vector.memset(vp4[:, :, :, DH:DH + 1], 1.0)
            nc.vector.memset(vp4[:, :, :, DH + 1:DH + 2], 0.0)
            for i in range(WARM):
                warm = ps.tile([128, 512], F32, tag="small", bufs=2,
                               name=f"warm{i}")
                nc.tensor.matmul(warm[:], wa_t[:], wb_t[:],
                                 start=True, stop=True)

            # ---- input DMAs: k-half quarters, priority q > k > v ----
            def xq4(which, sc, kh):
                xd = {"q": xq_d, "k": xk_d, "v": xv_d}[which]
                dst = x_t[(which, sc, kh)][:]
                src = xd[:, sc * KT * 512 + kh * HK:
                         sc * KT * 512 + (kh + 1) * HK]
                return dst, src

            # Pin each input DMA's *scheduling-pass* timestamp to its
            # realistic completion time (the scheduler's DMA model is
            # optimistic, which otherwise orders consumers of late DMAs
            # ahead of ready work in the engine FIFOs).
            def pdma(eng, dst, src, ms):
                with tc.tile_wait_until(ms):
                    eng.dma_start(dst, src)

            pdma(nc.gpsimd, wq_t[0][:], wq_d[:, 0:2 * KT * 128], 0.0025)
            pdma(nc.gpsimd, wk_t[0][:], wk_d[:, 0:2 * KT * 128], 0.004)
            pdma(nc.gpsimd, bqk_t[:], bqk_d, 0.0045)
            pdma(nc.gpsimd, bv_t[:], bv_d, 0.0046)
            pdma(nc.gpsimd, wq_t[1][:], wq_d[:, 2 * KT * 128:], 0.0065)
            pdma(nc.gpsimd, wk_t[1][:], wk_d[:, 2 * KT * 128:], 0.008)
            xq_ms = {("q", 0): 0.004, ("q", 1): 0.005,
                     ("k", 0): 0.0065, ("k", 1): 0.008}
            for which in ("q", "k"):
                for sc in (0, 1):
                    ms = xq_ms[(which, sc)]
                    pdma(nc.sync, *xq4(which, sc, 0), ms)
                    pdma(nc.scalar, *xq4(which, sc, 1), ms)
            pdma(nc.sync, *xq4("v", 0, 0), 0.012)
            pdma(nc.scalar, *xq4("v", 1, 0), 0.012)
            pdma(nc.gpsimd, *xq4("v", 0, 1), 0.0125)
            pdma(nc.gpsimd, *xq4("v", 1, 1), 0.013)
            pdma(nc.sync, wv_t[0][:], wv_d[:, 0:KT // 2 * GH], 0.0135)
            pdma(nc.scalar, wv_t[1][:], wv_d[:, KT // 2 * GH:], 0.0135)
            # preload ACT exp table (after the scalar-ring DMA kicks)
            nc.scalar.activation(dummy_t[:], wb_t[0:1, 0:8],
                                 mybir.ActivationFunctionType.Exp, scale=1.0)

            # rearranged views: (which, sc, c2) -> moving/stationary k-pair
            def wqk_pair(w_t, ot, c2):
                half = w_t[ot // 2]
                base = (ot % 2) * KT * 128
                return half[:, base:base + KT * 128].rearrange(
                    "p (k o) -> p k o", o=128)[:, 2 * c2:2 * c2 + 2, :]

            def x_pair(which, sc, c2):
                kh, c = c2 // 2, c2 % 2
                return x_t[(which, sc, kh)][:].rearrange(
                    "p (k s) -> p k s", s=512)[:, 2 * c:2 * c + 2, :]

            def wv_pair(c2):
                kh, c = c2 // 2, c2 % 2
                return wv_t[kh][:].rearrange(
                    "p (k o) -> p k o", o=GH)[:, 2 * c:2 * c + 2, :]

            vp3 = vp_t[:].rearrange("p (k m) -> p k m", m=VW)

            # ---- engine balancing for exp tiles ----
            exp_state = {"acc": 0}

            def exp_engine():
                exp_state["acc"] += ACT_EXPS
                if exp_state["acc"] >= 64:
                    exp_state["acc"] -= 64
                    return "act"
                return "dve"

            pt_tiles = {}

            def pt_tile(n, c):
                if (n, c) not in pt_tiles:
                    pt_tiles[(n, c)] = sb.tile([128, 2048], FP8, tag="pt",
                                               bufs=32, name=f"pt_{n}_{c}")
                return pt_tiles[(n, c)]

            # ---- stage helpers ----
            def proj_qk(which, ot):
                w_t = wq_t if which == "q" else wk_t
                dst = qp_t[ot] if which == "q" else kp_t[ot]
                wi = 0 if which == "q" else 1
                bias = bqk_t[:, wi * OT + ot:wi * OT + ot + 1]
                pp = ps.tile([128, 1024], F32, tag="apt", bufs=3,
                             name=f"pp{which}{ot}")
                for sc in range(2):
                    for c2 in range(KT // 2):
                        nc.tensor.matmul(
                            pp[:, sc * 512:(sc + 1) * 512],
                            wqk_pair(w_t, ot, c2),
                            x_pair(which, sc, c2),
                            start=(c2 == 0), stop=(c2 == KT // 2 - 1),
                            perf_mode=DR)
                nc.scalar.activation(
                    dst[:], pp[:],
                    mybir.ActivationFunctionType.Relu,
                    bias=bias, scale=1.0)

            def proj_v(st):
                sc, j = st // 4, st % 4
                pp = ps.tile([128, 512], F32, tag="small", bufs=2,
                             name=f"ppv{st}")
                if bias_v:
                    nc.tensor.matmul(pp[:], wa_t[0:1, :], bv_t[:],
                                     start=True, stop=False)
                for c2 in range(KT // 2):
                    nc.tensor.matmul(
                        pp[:],
                        x_pair("v", sc, c2)[:, :, j * 128:(j + 1) * 128],
                        wv_pair(c2),
                        start=(c2 == 0 and not bias_v),
                        stop=(c2 == KT // 2 - 1),
                        perf_mode=DR)
                # evac with relu into the strided fp8 v layout (cols 0..63)
                vdst = vp4[:, st, :, 0:DH]
                psrc = pp[:].rearrange("p (n c) -> p n c", c=DH)
                nc.vector.tensor_scalar(
                    vdst, psrc, 0.0, None, mybir.AluOpType.max)

            def alpha_pair(t, k):
                """alpha + exp for heads (2t, 2t+1), sk-tile k: two K=64
                matmuls on disjoint PE row-groups run concurrently."""
                apts = []
                for h in range(2):
                    apt = ps.tile([128, 1024], F32, tag="apt", bufs=3,
                                  name=f"alp_{2 * t + h}_{k}")
                    apts.append(apt)
                for h in range(2):
                    for qc in range(2):
                        pr = slice(h * 64, h * 64 + 64)
                        nc.tensor.matmul(
                            apts[h][:, qc * 512:(qc + 1) * 512],
                            kp_t[t][pr, k * 128:(k + 1) * 128],
                            qp_t[t][pr, qc * 512:(qc + 1) * 512],
                            start=True, stop=True)
                for h in range(2):
                    n = 2 * t + h
                    pt = pt_tile(n, k // 2)
                    half = pt[:, (k % 2) * 1024:(k % 2) * 1024 + 1024]
                    if exp_engine() == "act":
                        nc.scalar.activation(
                            half, apts[h][:],
                            mybir.ActivationFunctionType.Exp,
                            bias=nshift_t[:], scale=SCALE)
                    else:
                        nc.vector.tensor_scalar(
                            half.bitcast(I8), apts[h][:],
                            SCHR_A, SCHR_B,
                            mybir.AluOpType.mult, mybir.AluOpType.add)

            av_state = {"i": 0}
            hs_tiles = {}

            def av_qc(n, qc):
                if n not in hs_tiles:
                    hs_tiles[n] = sb.tile([DH + 1, S], BF16, tag="hid",
                                          bufs=3, name=f"hid_{n}")
                hs = hs_tiles[n]
                av = ps.tile([128, 512], F32, tag="small", bufs=2,
                             name=f"av_{n}_{qc}")
                for c2 in range(KT // 2):
                    pt = pt_tile(n, c2)
                    ptm = pt[:].rearrange("p (two q) -> p two q", two=2)
                    nc.tensor.matmul(
                        av[0:PVW, :],
                        vp3[:, 2 * c2:2 * c2 + 2,
                            n * PVW:(n + 1) * PVW],
                        ptm[:, :, qc * 512:(qc + 1) * 512],
                        start=(c2 == 0), stop=(c2 == KT // 2 - 1),
                        perf_mode=DR)
                i = av_state["i"]
                av_state["i"] += 1
                dst = hs[:, qc * 512:(qc + 1) * 512]
                if i % 2 == 0:
                    nc.scalar.copy(dst, av[0:DH + 1, :])
                else:
                    nc.vector.tensor_copy(dst, av[0:DH + 1, :])
                eng = nc.sync if i % 2 == 0 else nc.gpsimd
                eng.dma_start(
                    hid_d[n * (DH + 1):(n + 1) * (DH + 1),
                          qc * 512:(qc + 1) * 512],
                    dst)

            # ---- emission schedule: alpha mini-blocks with DR fillers ----
            proj_qk("q", 0)
            proj_qk("k", 0)
            proj_qk("q", 1)
            proj_qk("k", 1)
            for k in range(KT):           # t0 alphas, proj_v fillers
                alpha_pair(0, k)
                proj_v(k)
            proj_qk("q", 2)
            proj_qk("k", 2)
            for k in range(KT):           # t1 alphas, av fillers
                alpha_pair(1, k)
                if k == 2:
                    av_qc(0, 0)
                    av_qc(0, 1)
                elif k == 6:
                    av_qc(1, 0)
                    av_qc(1, 1)
            proj_qk("q", 3)
            proj_qk("k", 3)
            for k in range(KT):           # t2 alphas
                alpha_pair(2, k)
                if k == 2:
                    av_qc(2, 0)
                    av_qc(2, 1)
                elif k == 6:
                    av_qc(3, 0)
                    av_qc(3, 1)
            for k in range(KT):           # t3 alphas
                alpha_pair(3, k)
                if k == 2:
                    av_qc(4, 0)
                    av_qc(4, 1)
                elif k == 6:
                    av_qc(5, 0)
                    av_qc(5, 1)
            for n in (6, 7):
                av_qc(n, 0)
                av_qc(n, 1)

    nc.compile()
    return nc


_NC_CACHE = {}


def _get_nc(mode, bias_v=False):
    key = (mode, bias_v)
    if key not in _NC_CACHE:
        _NC_CACHE[key] = build(mode, bias_v)
    return _NC_CACHE[key]


def _prep_inputs(inputs, mode):
    q = np.asarray(inputs["query"], np.float32)
    k = np.asarray(inputs["key"], np.float32)
    v = np.asarray(inputs["value"], np.float32)
    Wq = np.asarray(inputs["Wq"], np.float32)
    Wk = np.asarray(inputs["Wk"], np.float32)
    Wv = np.asarray(inputs["Wv"], np.float32)
    bq = np.asarray(inputs["bq"], np.float32)
    bk = np.asarray(inputs["bk"], np.float32)
    bv = np.asarray(inputs["bv"], np.float32)

    def xprep(x, b):
        # [H, S] -> [p, sc, k, s'] -> [128, 8192] fp8
        xt = np.ascontiguousarray(x[b].T).astype(E4NP)
        return np.ascontiguousarray(
            xt.reshape(KT, 128, 2, 512).transpose(1, 2, 0, 3)
        ).reshape(128, 2 * KT * 512)

    def wqk_prep(W, sl):
        # W[sl].T: [H, GH] -> [p, ot, k, o'] -> [128, 4096] fp8
        wt = np.ascontiguousarray(W[sl, :].T).astype(E4NP)
        return np.ascontiguousarray(
            wt.reshape(KT, 128, OT, 128).transpose(1, 2, 0, 3)
        ).reshape(128, OT * KT * 128)

    def wv_prep(W, sl):
        # W[sl].T: [H, GH] -> [p, k, o] -> [128, 4096] fp8
        wt = np.ascontiguousarray(W[sl, :].T).astype(E4NP)
        return np.ascontiguousarray(
            wt.reshape(KT, 128, GH).transpose(1, 0, 2)
        ).reshape(128, KT * GH)

    xq = [xprep(q, b) for b in range(B)]
    xk = [xprep(k, b) for b in range(B)]
    xv = [xprep(v, b) for b in range(B)]
    in_maps = []
    for c in range(NCORES):
        b, g = c // GROUPS, c % GROUPS
        sl = slice(g * GH, (g + 1) * GH)
        bqk = np.stack([bq[sl].reshape(OT, 128).T,
                        bk[sl].reshape(OT, 128).T], 1).reshape(128, 2 * OT)
        in_maps.append({
            "xq": xq[b], "xk": xk[b], "xv": xv[b],
            "wq": wqk_prep(Wq, sl),
            "wk": wqk_prep(Wk, sl),
            "wv": wv_prep(Wv, sl),
            "bqk": np.ascontiguousarray(bqk, dtype=np.float32),
            "bv": np.ascontiguousarray(bv[None, sl]).astype(E4NP),
        })
    return in_maps


def run(inputs, mode=MODE, trace=False):
    bias_v = bool(np.any(np.asarray(inputs["bv"], np.float32)))
    nc = _get_nc(mode, bias_v)
    in_maps = _prep_inputs(inputs, mode)
    res = bass_utils.run_bass_kernel_spmd(
        nc, in_maps, core_ids=list(range(NCORES)), trace=trace)

    masks = np.asarray(inputs["masks"], np.float32)
    query = np.asarray(inputs["query"], np.float32)
    out = np.empty((B, S, H), np.float32)
    for c in range(NCORES):
        b, g = c // GROUPS, c % GROUPS
        hid = np.asarray(res.results[c]["hid"],
                         dtype=np.float32).reshape(HL, DH + 1, S)
        hT = hid[:, :DH, :]                      # (HL, DH, S)
        se = hid[:, DH, :]                       # (HL, S)
        blk = (hT / se[:, None, :]).transpose(2, 0, 1).reshape(S, GH)
        out[b, :, g * GH:(g + 1) * GH] = blk
    out = out * masks[:, :, None] + query
    return out, res


def kernel(**inputs) -> np.ndarray:
    out, _ = run(inputs)
    return out
